# revision 23
# baseline (speedup 1.0000x reference)
"""AdaConv2d (per-sample masked 3x3 conv) on 8 TRN2 NeuronCores.

Strategy (data-parallel, per sharding hint):
  - 64 samples sharded 8-per-core; kernel_base/kernel_mask replicated.
  - Two samples share one 128-partition SBUF tile: sample A's padded
    image (one input channel per partition) in partitions 0-63, sample
    B's in 64-127. No shifted copies -> input DMA is 1x the image.
  - Each of the 9 conv taps is one K=64 matmul; per (tap, 4-row block)
    four M=64 matmuls run concurrently on the four 64x64 quadrants of
    the PE array (tile_position auto-derived from base partitions), so
    all 16384 MACs/cycle are live on every pass (PE roofline for this
    decomposition ~94us; measured MM cadence sits at the issue floor).
  - Per-sample kernels (kernel_base * kernel_mask[label], bf16, lhsT
    layout) are precomputed on the host.
  - Scheduling notes (trace-driven; v1=117.6us, final ~114.3us):
    * All DMAs share a ~9-slot completion-semaphore rotation assigned
      in global issue order; before reusing a slot the issuing queue
      WAITS for the slot's previous transfer. Front-loaded input
      issues therefore make output DMAs dribble out 10-20us late
      (head-of-line on sync). Countermeasures: ONE output DMA per
      round (not two - slot demand doubled was v4/v5's regression),
      16 rounds of stage-buffer slack to absorb the residual drift
      (v1's 12 was marginal, my 6 stalled the PE), pair-0's input
      chunks split across BOTH hw wire queues (scalar+sync) so the
      head backlog drains in half the time, and pairs 1-3 prefetched
      one 8-row chunk per round from the previous pair's loop instead
      of front-loaded.
    * Only sync (SP) and scalar (ACT) have hardware DGE queues;
      gpsimd DMA is software-DGE and adds a ~10.8us teardown drain.
    * Only the LAST round's output is split into two DMAs (after each
      CAST, on the two hw queues) to shave ~1us off the exposed tail.
    * PE warmup (HAM clock-gate needs ~3.4us of activity) memsets its
      dummy tile on the vector queue, which clears its preamble first.
    * The paced prefetch MUST cover all NCH chunks in ROUNDS rounds:
      XROWS has exactly 14 chunks (last one 10 rows). A 15-chunk grid
      silently left rows 112-113 unwritten for pairs 1-3 (read of
      uninitialized SBUF in their last round; rel err 9.9e-2).
    * Head is HBM-bound: issue->first-bytes is ~2.8us and round 0
      needs ~0.83MB (weights + 12 rows) across both wire queues, so
      the first real matmul can't start before ~11us; the warmup
      dovetails into that window for free.
"""
import numpy as np
import ml_dtypes

import concourse.bass as bass  # noqa: F401  (registers engines)
import concourse.tile as tile
from concourse import bacc, mybir
from concourse.bass_utils import run_bass_kernel_spmd

NCORES = 8
SPC = 8            # samples per core
PAIRS = SPC // 2   # two samples share one 128-partition tile
H = W = 112
IC = OC = 64
ND = 4             # demographic groups
PW = H + 2         # padded width/height
PHW = PW * PW
RB = 4             # output rows per matmul block
N = RB * W         # 448 columns per matmul (one PSUM bank)
ROUNDS = H // (2 * RB)   # 14 rounds of (even, odd) blocks per sample
NTAP = 9
FUSE_EPOCH = 9
F32 = mybir.dt.float32
BF16 = mybir.dt.bfloat16

# x chunk boundaries (padded-row units) for pairs >= 1: exactly
# ROUNDS(=14) chunks so the one-per-round paced prefetch covers the
# whole image (last chunk 10 rows); pair 0 gets two 6-row leading
# chunks so round 0's rows land soonest
XROWS = list(range(0, 105, 8)) + [PW]
XROWS0 = [0, 6, 12, 20, 28, 36, 44, 52, 60, 68, 76, 84, 92, 100, 108, PW]
# 32 dummy matmuls (~3.4us) reliably span a full HAM activity window;
# 28 flipped the clock-gate 4-6us late on ~2/3 of runs (cold stream)
NWARM = 32

_CACHE = {}


def _build():
    nc = bacc.Bacc("TRN2", target_bir_lowering=False, debug=False,
                   num_devices=NCORES)
    xs = nc.dram_tensor("xs", [PAIRS, 128, PHW], BF16,
                        kind="ExternalInput").ap()
    wd = nc.dram_tensor("wd", [PAIRS, 128, NTAP * 128], BF16,
                        kind="ExternalInput").ap()
    out = nc.dram_tensor("out", [PAIRS, ROUNDS, 2, 2, OC, N], BF16,
                         kind="ExternalOutput").ap()

    # combined view: per partition (blk*oc), [sample-in-pair, rb*w]
    ovc = out.rearrange("pr r b k oc f -> pr r (k oc) b f")
    # split view for the last round's two DMAs
    ovs = out.rearrange("pr r b k oc f -> pr r b (k oc) f")
    wdr = wd.rearrange("pr p (j m) -> pr p j m", m=128)

    with tile.TileContext(nc) as tc:
        with (
            tc.tile_pool(name="xp", bufs=3) as xp,
            tc.tile_pool(name="wp", bufs=2) as wp,
            tc.tile_pool(name="stage", bufs=16) as stp,
            tc.tile_pool(name="psum", bufs=4, space="PSUM") as pp,
            tc.tile_pool(name="warm", bufs=1) as wmp,
        ):
            # warm up the PE HAM clock-gate with dummy matmuls while
            # pair 0's inputs are in flight; memset on the vector queue
            # (earliest past its preamble). The scratch PSUM borrows a
            # generation of the main pool.
            warm = wmp.tile([128, 128], BF16, name="warm", tag="warm")
            nc.vector.memset(warm[:], 0)
            psW = pp.tile([128, N], F32, name="psW", tag="psA")
            for _ in range(NWARM):
                nc.tensor.matmul(psW[0:64, 0:128], warm[:, 0:64], warm[:],
                                 start=True, stop=True)

            xts, wts = {}, {}

            def xt_for(p):
                if p not in xts:
                    t = xp.tile([128, PHW], BF16, name="xt", tag="xt")
                    xts[p] = (t, t.rearrange("p (r c) -> p r c", c=PW))
                return xts[p]

            def wt_for(p):
                if p not in wts:
                    t = wp.tile([128, NTAP * 128], BF16, name="wt",
                                tag="wt")
                    wts[p] = (t, t.rearrange("p (j m) -> p j m", m=128))
                return wts[p]

            def xchunk(p, q, eng, rows=XROWS):
                xt, _ = xt_for(p)
                qs, qe = rows[q] * PW, rows[q + 1] * PW
                eng.dma_start(xt[:, qs:qe], xs[p][:, qs:qe])

            NCH = len(XROWS) - 1    # 14 chunks per pair

            # pair 0 head, split across both hw wire queues: q0 on
            # scalar and q1 on sync land round 0's rows in parallel;
            # weights follow on sync in three 3-tap slices so round
            # 0's first LDWEIGHTS waits a 98KB slice, not 295KB
            xchunk(0, 0, nc.scalar, XROWS0)
            xchunk(0, 1, nc.sync, XROWS0)
            _, w30 = wt_for(0)
            for jt in range(0, NTAP, 3):
                nc.sync.dma_start(w30[:, jt:jt + 3, :],
                                  wdr[0][:, jt:jt + 3, :])
            for q in (2, 3, 4, 5, 6, 8, 10, 12, 14):
                xchunk(0, q, nc.scalar, XROWS0)
            for q in (7, 9, 11, 13):
                xchunk(0, q, nc.sync, XROWS0)

            for pr in range(PAIRS):
                _, x3 = xt_for(pr)
                _, w3 = wt_for(pr)

                for rnd in range(ROUNDS):
                    last_round = pr == PAIRS - 1 and rnd == ROUNDS - 1
                    psA = pp.tile([128, N], F32, name="psA", tag="psA")
                    psB = pp.tile([128, N], F32, name="psB", tag="psB")
                    for j in range(NTAP):
                        dy, dx = divmod(j, 3)
                        first, last = (j == 0), (j == NTAP - 1)
                        for blk in range(2):
                            r0 = rnd * 2 * RB + blk * RB + dy
                            pc = blk * 64
                            rA = x3[0:64, r0:r0 + RB, dx:dx + W]
                            rB = x3[64:128, r0:r0 + RB, dx:dx + W]
                            mmA = (psA[pc:pc + 64, :],
                                   w3[0:64, j, pc:pc + 64], rA)
                            mmB = (psB[pc:pc + 64, :],
                                   w3[64:128, j, pc:pc + 64], rB)
                            # last round: emit B first so B's stop-MMs
                            # retire first (B's CAST + scalar DMA are
                            # the critical tail path)
                            for o, l, r in ([mmB, mmA] if last_round
                                            else [mmA, mmB]):
                                nc.tensor.matmul(o, l, r,
                                                 start=first, stop=last)

                    st = stp.tile([128, 2, N], BF16, name="st", tag="st")
                    if last_round:
                        # split the exposed tail: B's half CASTs and
                        # ships first (scalar queue), A's follows on
                        # sync so the two transfers overlap
                        nc.vector.tensor_copy(st[:, 1, :], psB[:])
                        nc.scalar.dma_start(ovs[pr, rnd, 1], st[:, 1, :])
                        nc.vector.tensor_copy(st[:, 0, :], psA[:])
                        nc.sync.dma_start(ovs[pr, rnd, 0], st[:, 0, :])
                    else:
                        nc.vector.tensor_copy(st[:, 0, :], psA[:])
                        nc.vector.tensor_copy(st[:, 1, :], psB[:])
                        nc.sync.dma_start(ovc[pr, rnd], st[:])

                    # paced prefetch of the next pair's inputs on the
                    # scalar queue: weights at round 0, one 8-row x
                    # chunk per round (0.93us wire vs 1.71us round ->
                    # no issued-but-untransferred backlog builds up)
                    if pr + 1 < PAIRS:
                        if rnd == 0:
                            wtn, _ = wt_for(pr + 1)
                            nc.scalar.dma_start(wtn[:], wd[pr + 1])
                            xchunk(pr + 1, 0, nc.scalar)
                        elif rnd < NCH:
                            xchunk(pr + 1, rnd, nc.scalar)

    nc.compile()
    return nc


def get_nc():
    if "nc" not in _CACHE:
        _CACHE["nc"] = _build()
    return _CACHE["nc"]


def make_in_maps(x, kernel_base, kernel_mask, demog_label, epoch):
    kb = np.asarray(kernel_base, dtype=np.float32)
    km = np.asarray(kernel_mask, dtype=np.float32)
    labels = np.asarray(demog_label).astype(np.int64)
    if int(np.asarray(epoch)) >= FUSE_EPOCH:
        labels = np.zeros_like(labels)

    B = labels.shape[0]
    # padded bf16 image per sample (layout only); pairs share a tile
    xb = np.asarray(x, dtype=np.float32).astype(ml_dtypes.bfloat16)
    xpad = np.zeros((B, IC, PW, PW), dtype=ml_dtypes.bfloat16)
    xpad[:, :, 1:H + 1, 1:W + 1] = xb
    xfull = xpad.reshape(B // 2, 128, PHW)

    # per-sample lhsT weights [ic, tap, oc], duplicated across the two
    # 64-col halves of the PE array
    kbT = kb.reshape(OC, IC, NTAP).transpose(1, 2, 0)   # [ic, j, oc]
    km9 = km.reshape(ND, IC, NTAP)                      # [d, ic, j]
    # ws[d, ic, j, oc] = kb[oc, ic, j] * km[d, ic, j]
    ws = kbT[None] * km9[:, :, :, None]                 # [d, ic, j, oc]
    wdup = np.concatenate([ws, ws], axis=3)             # [d, ic, j, 128]
    wdup = wdup.reshape(ND, IC, NTAP * 128).astype(ml_dtypes.bfloat16)

    in_maps = []
    for c in range(NCORES):
        lab = labels[c * SPC:(c + 1) * SPC]
        wdc = np.zeros((PAIRS, 128, NTAP * 128), dtype=ml_dtypes.bfloat16)
        for p in range(PAIRS):
            wdc[p, 0:IC] = wdup[lab[2 * p]]
            wdc[p, IC:] = wdup[lab[2 * p + 1]]
        in_maps.append({
            "xs": np.ascontiguousarray(
                xfull[c * PAIRS:(c + 1) * PAIRS]),
            "wd": wdc,
        })
    return in_maps


def kernel(x, kernel_base, kernel_mask, demog_label, epoch):
    nc = get_nc()
    in_maps = make_in_maps(x, kernel_base, kernel_mask, demog_label, epoch)
    res = run_bass_kernel_spmd(nc, in_maps, list(range(NCORES)))
    outs = []
    for c in range(NCORES):
        raw = res.results[c]["out"].astype(np.float32)
        # [PAIRS, ROUNDS, b, blk, OC, RB, W] -> [PAIRS, b, OC, R, blk, RB, W]
        raw = raw.reshape(PAIRS, ROUNDS, 2, 2, OC, RB, W)
        raw = raw.transpose(0, 2, 4, 1, 3, 5, 6)
        outs.append(raw.reshape(SPC, OC, H, W))
    return np.concatenate(outs, axis=0)


# revision 24
# speedup vs baseline: 1.0170x; 1.0170x over previous
"""AdaConv2d (per-sample masked 3x3 conv) on 8 TRN2 NeuronCores.

Strategy (data-parallel, per sharding hint):
  - 64 samples sharded 8-per-core; kernel_base/kernel_mask replicated.
  - Two samples share one 128-partition SBUF tile: sample A's padded
    image (one input channel per partition) in partitions 0-63, sample
    B's in 64-127. No shifted copies -> input DMA is 1x the image.
  - Each of the 9 conv taps is one K=64 matmul; per (tap, 4-row block)
    four M=64 matmuls run concurrently on the four 64x64 quadrants of
    the PE array (tile_position auto-derived from base partitions), so
    all 16384 MACs/cycle are live on every pass (PE roofline for this
    decomposition ~94us; measured MM cadence sits at the issue floor).
  - Per-sample kernels (kernel_base * kernel_mask[label], bf16, lhsT
    layout) are precomputed on the host.
  - Scheduling notes (trace-driven; v1=117.6us, final ~114.3us):
    * All DMAs share a ~9-slot completion-semaphore rotation assigned
      in global issue order; before reusing a slot the issuing queue
      WAITS for the slot's previous transfer. Front-loaded input
      issues therefore make output DMAs dribble out 10-20us late
      (head-of-line on sync). Countermeasures: ONE output DMA per
      round (not two - slot demand doubled was v4/v5's regression),
      16 rounds of stage-buffer slack to absorb the residual drift
      (v1's 12 was marginal, my 6 stalled the PE), pair-0's input
      chunks split across BOTH hw wire queues (scalar+sync) so the
      head backlog drains in half the time, and pairs 1-3 prefetched
      one 8-row chunk per round from the previous pair's loop instead
      of front-loaded.
    * Only sync (SP) and scalar (ACT) have hardware DGE queues;
      gpsimd DMA is software-DGE and adds a ~10.8us teardown drain.
    * Only the LAST round's output is split into two DMAs (after each
      CAST, on the two hw queues) to shave ~1us off the exposed tail.
    * PE warmup (HAM clock-gate needs ~3.4us of activity) memsets its
      dummy tile on the vector queue, which clears its preamble first.
    * The paced prefetch MUST cover all NCH chunks in ROUNDS rounds:
      XROWS has exactly 14 chunks (last one 10 rows). A 15-chunk grid
      silently left rows 112-113 unwritten for pairs 1-3 (read of
      uninitialized SBUF in their last round; rel err 9.9e-2).
    * Head is HBM-bound: issue->first-bytes is ~2.8us and round 0
      needs ~0.83MB (weights + 12 rows) across both wire queues, so
      the first real matmul can't start before ~11us; the warmup
      dovetails into that window for free.
"""
import numpy as np
import ml_dtypes

import concourse.bass as bass  # noqa: F401  (registers engines)
import concourse.tile as tile
from concourse import bacc, mybir
from concourse.bass_utils import run_bass_kernel_spmd

NCORES = 8
SPC = 8            # samples per core
PAIRS = SPC // 2   # two samples share one 128-partition tile
H = W = 112
IC = OC = 64
ND = 4             # demographic groups
PW = H + 2         # padded width/height
PHW = PW * PW
RB = 4             # output rows per matmul block
N = RB * W         # 448 columns per matmul (one PSUM bank)
ROUNDS = H // (2 * RB)   # 14 rounds of (even, odd) blocks per sample
NTAP = 9
FUSE_EPOCH = 9
F32 = mybir.dt.float32
BF16 = mybir.dt.bfloat16

# x chunk boundaries (padded-row units) for pairs >= 1: exactly
# ROUNDS(=14) chunks so the one-per-round paced prefetch covers the
# whole image (last chunk 10 rows); pair 0 gets two 6-row leading
# chunks so round 0's rows land soonest
XROWS = list(range(0, 105, 8)) + [PW]
XROWS0 = [0, 6, 12, 20, 28, 36, 44, 52, 60, 68, 76, 84, 92, 100, 108, PW]
# 32 dummy matmuls (~3.4us) reliably span a full HAM activity window;
# 28 flipped the clock-gate 4-6us late on ~2/3 of runs (cold stream)
NWARM = 32

_CACHE = {}


def _build():
    nc = bacc.Bacc("TRN2", target_bir_lowering=False, debug=False,
                   num_devices=NCORES)
    xs = nc.dram_tensor("xs", [PAIRS, 128, PHW], BF16,
                        kind="ExternalInput").ap()
    wd = nc.dram_tensor("wd", [PAIRS, 128, NTAP * 128], BF16,
                        kind="ExternalInput").ap()
    out = nc.dram_tensor("out", [PAIRS, ROUNDS, 2, 2, OC, N], BF16,
                         kind="ExternalOutput").ap()

    # combined view: per partition (blk*oc), [sample-in-pair, rb*w]
    ovc = out.rearrange("pr r b k oc f -> pr r (k oc) b f")
    # split view for the last round's two DMAs
    ovs = out.rearrange("pr r b k oc f -> pr r b (k oc) f")
    wdr = wd.rearrange("pr p (j m) -> pr p j m", m=128)

    with tile.TileContext(nc) as tc:
        with (
            tc.tile_pool(name="xp", bufs=3) as xp,
            tc.tile_pool(name="wp", bufs=2) as wp,
            tc.tile_pool(name="stage", bufs=16) as stp,
            tc.tile_pool(name="psum", bufs=4, space="PSUM") as pp,
            tc.tile_pool(name="warm", bufs=1) as wmp,
        ):
            # warm up the PE HAM clock-gate with dummy matmuls while
            # pair 0's inputs are in flight; memset on the vector queue
            # (earliest past its preamble). The scratch PSUM borrows a
            # generation of the main pool.
            warm = wmp.tile([128, 128], BF16, name="warm", tag="warm")
            nc.vector.memset(warm[:], 0)
            psW = pp.tile([128, N], F32, name="psW", tag="psA")
            for _ in range(NWARM):
                nc.tensor.matmul(psW[0:64, 0:128], warm[:, 0:64], warm[:],
                                 start=True, stop=True)

            xts, wts = {}, {}

            def xt_for(p):
                if p not in xts:
                    t = xp.tile([128, PHW], BF16, name="xt", tag="xt")
                    xts[p] = (t, t.rearrange("p (r c) -> p r c", c=PW))
                return xts[p]

            def wt_for(p):
                if p not in wts:
                    t = wp.tile([128, NTAP * 128], BF16, name="wt",
                                tag="wt")
                    wts[p] = (t, t.rearrange("p (j m) -> p j m", m=128))
                return wts[p]

            def xchunk(p, q, eng, rows=XROWS):
                xt, _ = xt_for(p)
                qs, qe = rows[q] * PW, rows[q + 1] * PW
                eng.dma_start(xt[:, qs:qe], xs[p][:, qs:qe])

            NCH = len(XROWS) - 1    # 14 chunks per pair

            # pair 0 head: first two (6-row) chunks on scalar so round
            # 0's rows land earliest, rest alternating scalar/sync so
            # the two wire queues drain the burst concurrently;
            # weights on sync. (Splitting wd0 per-tap and moving q1 to
            # sync was tried: it delayed round 0 ~1.4us and pushed the
            # HAM flip into the cold-phase regime - net +3.5us.)
            xchunk(0, 0, nc.scalar, XROWS0)
            xchunk(0, 1, nc.scalar, XROWS0)
            wt0, _ = wt_for(0)
            nc.sync.dma_start(wt0[:], wd[0])
            for q in range(2, len(XROWS0) - 1):
                xchunk(0, q, nc.scalar if q % 2 == 0 else nc.sync,
                       XROWS0)

            for pr in range(PAIRS):
                _, x3 = xt_for(pr)
                _, w3 = wt_for(pr)

                for rnd in range(ROUNDS):
                    last_round = pr == PAIRS - 1 and rnd == ROUNDS - 1
                    psA = pp.tile([128, N], F32, name="psA", tag="psA")
                    psB = pp.tile([128, N], F32, name="psB", tag="psB")
                    for j in range(NTAP):
                        dy, dx = divmod(j, 3)
                        first, last = (j == 0), (j == NTAP - 1)
                        for blk in range(2):
                            r0 = rnd * 2 * RB + blk * RB + dy
                            pc = blk * 64
                            rA = x3[0:64, r0:r0 + RB, dx:dx + W]
                            rB = x3[64:128, r0:r0 + RB, dx:dx + W]
                            mmA = (psA[pc:pc + 64, :],
                                   w3[0:64, j, pc:pc + 64], rA)
                            mmB = (psB[pc:pc + 64, :],
                                   w3[64:128, j, pc:pc + 64], rB)
                            # last round: emit B first so B's stop-MMs
                            # retire first (B's CAST + scalar DMA are
                            # the critical tail path)
                            for o, l, r in ([mmB, mmA] if last_round
                                            else [mmA, mmB]):
                                nc.tensor.matmul(o, l, r,
                                                 start=first, stop=last)

                    st = stp.tile([128, 2, N], BF16, name="st", tag="st")
                    if last_round:
                        # split the exposed tail: B's half CASTs and
                        # ships first (scalar queue), A's follows on
                        # sync so the two transfers overlap
                        nc.vector.tensor_copy(st[:, 1, :], psB[:])
                        nc.scalar.dma_start(ovs[pr, rnd, 1], st[:, 1, :])
                        nc.vector.tensor_copy(st[:, 0, :], psA[:])
                        nc.sync.dma_start(ovs[pr, rnd, 0], st[:, 0, :])
                    else:
                        nc.vector.tensor_copy(st[:, 0, :], psA[:])
                        nc.vector.tensor_copy(st[:, 1, :], psB[:])
                        nc.sync.dma_start(ovc[pr, rnd], st[:])

                    # paced prefetch of the next pair's inputs on the
                    # scalar queue: weights at round 0, one 8-row x
                    # chunk per round (0.93us wire vs 1.71us round ->
                    # no issued-but-untransferred backlog builds up)
                    if pr + 1 < PAIRS:
                        if rnd == 0:
                            wtn, _ = wt_for(pr + 1)
                            nc.scalar.dma_start(wtn[:], wd[pr + 1])
                            xchunk(pr + 1, 0, nc.scalar)
                        elif rnd < NCH:
                            xchunk(pr + 1, rnd, nc.scalar)

    nc.compile()
    return nc


def get_nc():
    if "nc" not in _CACHE:
        _CACHE["nc"] = _build()
    return _CACHE["nc"]


def make_in_maps(x, kernel_base, kernel_mask, demog_label, epoch):
    kb = np.asarray(kernel_base, dtype=np.float32)
    km = np.asarray(kernel_mask, dtype=np.float32)
    labels = np.asarray(demog_label).astype(np.int64)
    if int(np.asarray(epoch)) >= FUSE_EPOCH:
        labels = np.zeros_like(labels)

    B = labels.shape[0]
    # padded bf16 image per sample (layout only); pairs share a tile
    xb = np.asarray(x, dtype=np.float32).astype(ml_dtypes.bfloat16)
    xpad = np.zeros((B, IC, PW, PW), dtype=ml_dtypes.bfloat16)
    xpad[:, :, 1:H + 1, 1:W + 1] = xb
    xfull = xpad.reshape(B // 2, 128, PHW)

    # per-sample lhsT weights [ic, tap, oc], duplicated across the two
    # 64-col halves of the PE array
    kbT = kb.reshape(OC, IC, NTAP).transpose(1, 2, 0)   # [ic, j, oc]
    km9 = km.reshape(ND, IC, NTAP)                      # [d, ic, j]
    # ws[d, ic, j, oc] = kb[oc, ic, j] * km[d, ic, j]
    ws = kbT[None] * km9[:, :, :, None]                 # [d, ic, j, oc]
    wdup = np.concatenate([ws, ws], axis=3)             # [d, ic, j, 128]
    wdup = wdup.reshape(ND, IC, NTAP * 128).astype(ml_dtypes.bfloat16)

    in_maps = []
    for c in range(NCORES):
        lab = labels[c * SPC:(c + 1) * SPC]
        wdc = np.zeros((PAIRS, 128, NTAP * 128), dtype=ml_dtypes.bfloat16)
        for p in range(PAIRS):
            wdc[p, 0:IC] = wdup[lab[2 * p]]
            wdc[p, IC:] = wdup[lab[2 * p + 1]]
        in_maps.append({
            "xs": np.ascontiguousarray(
                xfull[c * PAIRS:(c + 1) * PAIRS]),
            "wd": wdc,
        })
    return in_maps


def kernel(x, kernel_base, kernel_mask, demog_label, epoch):
    nc = get_nc()
    in_maps = make_in_maps(x, kernel_base, kernel_mask, demog_label, epoch)
    res = run_bass_kernel_spmd(nc, in_maps, list(range(NCORES)))
    outs = []
    for c in range(NCORES):
        raw = res.results[c]["out"].astype(np.float32)
        # [PAIRS, ROUNDS, b, blk, OC, RB, W] -> [PAIRS, b, OC, R, blk, RB, W]
        raw = raw.reshape(PAIRS, ROUNDS, 2, 2, OC, RB, W)
        raw = raw.transpose(0, 2, 4, 1, 3, 5, 6)
        outs.append(raw.reshape(SPC, OC, H, W))
    return np.concatenate(outs, axis=0)
